# revision 11
# baseline (speedup 1.0000x reference)
"""Trainium2 Bass kernel for nn_AbsorberPathAggregator (v2).

Strategy: host-side path filtering (cutoff weight == 0 exactly for ~42% of
paths -> zero contribution, dropped exactly), survivors packed into 64-path
half-tiles batch-padded to 64, half-tiles distributed so each (core, batch)
owns at most one (seed-stable; a sum-merge variant handles up to two).

Device pipeline per core (W = T*128 path columns, T ~ 5):
  prepass: u = w1ab^T [ej;ek] (one matmul), v(e)+b1 (one matmul), rbf via
    broadcast-matmul + Square/Exp, geom MLP in bf16, gg2 = cw*(g3+b3) on DVE.
  e-loop over 40 folded e-pairs, software-pipelined across engines:
    scalar : h1 = silu(u (+) v_e)   -- the add rides the ACTIVATE bias port
    tensor : L2p = w2bd @ h1        (bf16, 640-wide moving)
    scalar : h2 = silu(L2p + b2)
    tensor : L3p = w3bd @ h2
    vector : contrib = L3p * gg2 ; slot cols = reduce_f(contrib) per half-tile
  tail: indirect-scatter slot blocks into batch-indexed DRAM accumulator,
    one ReduceScatter (each core receives exactly its 2 batches), then the
    out-projection on 2 batches and a direct store.
"""

import os

import numpy as np
import ml_dtypes

import concourse.bacc as bacc
import concourse.bass as bass
import concourse.mybir as mybir
import concourse.tile as tile
from concourse.bass_utils import run_bass_kernel_spmd

F32 = mybir.dt.float32
F32R = mybir.dt.float32r
BF16 = mybir.dt.bfloat16
I32 = mybir.dt.int32
NPBF16 = ml_dtypes.bfloat16

NCORES = 8
B = 16
BL = 2              # batches per core after ReduceScatter
NE = 80
S = 64
EP = NE // 2        # folded e-pairs
HF = 64             # paths per half-tile
SLOTC = 42          # 40 agg cols + Sgg col + norm col
ATOM = 128
RBF = 32
CUT = 5.0
RBF_SCALE = (RBF - 1) / CUT

_NC_CACHE = {}


def _bc_last(ap, n):
    """[...dims] -> [...dims, n] with 0-step last dim."""
    l = [list(x) for x in ap.ap]
    return bass.AP(ap.tensor, ap.offset, l + [[0, n]])


def build_nc(T: int, use_sum: bool) -> bass.Bass:
    nc = bacc.Bacc("TRN2", target_bir_lowering=False, debug=False,
                   num_devices=NCORES)
    AF = mybir.ActivationFunctionType
    ALU = mybir.AluOpType
    T2 = 2 * T
    W = T2 * HF

    def wch():
        out, k = [], 0
        while k < W:
            out.append((k, min(k + 512, W)))
            k += 512
        return out

    # ---- per-core inputs
    hjT_d = nc.dram_tensor("hjT", [ATOM, W], BF16, kind="ExternalInput")
    hkT_d = nc.dram_tensor("hkT", [ATOM, W], BF16, kind="ExternalInput")
    ejk_d = nc.dram_tensor("ejk", [64, W], BF16, kind="ExternalInput")
    r3_d = nc.dram_tensor("r3", [3, W], F32R, kind="ExternalInput")
    cos1_d = nc.dram_tensor("cos1", [1, W], F32, kind="ExternalInput")
    cw1_d = nc.dram_tensor("cw1", [1, W], F32R, kind="ExternalInput")
    sx_d = nc.dram_tensor("sx", [128, T2], I32, kind="ExternalInput")
    sxs_d = nc.dram_tensor("sxs", [128, T], I32, kind="ExternalInput")
    # ---- packed replicated params (one DMA each)
    wbf_d = nc.dram_tensor("wbf", [128, 1104], BF16, kind="ExternalInput")
    wfr_d = nc.dram_tensor("wfr", [128, 352], F32R, kind="ExternalInput")
    wf2_d = nc.dram_tensor("wf2", [128, 208], F32, kind="ExternalInput")
    # ---- output + collective buffers
    out_d = nc.dram_tensor("out", [BL * NE, S], F32, kind="ExternalOutput")
    aggin_d = nc.dram_tensor("aggin", [(B + 1) * 128, SLOTC], BF16)
    aggout_d = nc.dram_tensor("aggout", [BL * 128, SLOTC], BF16)

    with tile.TileContext(nc) as tc:
        with tc.tile_pool(name="const", bufs=1) as cp:
            def cl(dram, shape, dt, eng):
                t = cp.tile(shape, dt, tag=dram.name)
                eng.dma_start(t[:], dram[:])
                return t

            # ACT-table warmup: pull the Exp set in before anything needs it
            warm = cp.tile([1, 8], F32, tag="warm")
            nc.vector.memset(warm[:], 0.25)
            nc.scalar.activation(warm[0:1, 0:1], warm[0:1, 1:2], AF.Exp)

            # critical-path loads on the sync HWDGE ring
            ejk = cl(ejk_d, [64, W], BF16, nc.sync)
            wbf = cl(wbf_d, [128, 1104], BF16, nc.sync)
            r3 = cl(r3_d, [3, W], F32R, nc.sync)
            cw1 = cl(cw1_d, [1, W], F32R, nc.sync)
            hjA = cl(hjT_d, [ATOM, W], BF16, nc.sync)
            hkA = cl(hkT_d, [ATOM, W], BF16, nc.sync)
            # the rest via gpsimd SWDGE (its queue is idle here)
            wfr = cl(wfr_d, [128, 352], F32R, nc.gpsimd)
            wf2 = cl(wf2_d, [128, 208], F32, nc.gpsimd)
            cos1 = cl(cos1_d, [1, W], F32, nc.gpsimd)
            sx = cl(sx_d, [128, T2], I32, nc.gpsimd)
            sxs = cl(sxs_d, [128, T], I32, nc.gpsimd) if use_sum else None

            # views into the packed param tiles
            c = [0]

            def vw(t, rows, cols):
                a = t[0:rows, c[0]:c[0] + cols]
                c[0] += cols
                return a
            gw1a = vw(wbf, 128, 128)
            gw1b = vw(wbf, 128, 128)
            gw1c = vw(wbf, 97, 128)
            gw2 = vw(wbf, 128, 128)
            gw3 = vw(wbf, 128, 64)
            w2bd = vw(wbf, 128, 128)
            w3bd = vw(wbf, 128, 128)
            w1abD = vw(wbf, 64, 128)
            w1cA = vw(wbf, 33, 64)
            efA = vw(wbf, 33, NE)
            c = [0]
            ow1 = vw(wfr, 64, 128)
            ow2 = vw(wfr, 128, 64)
            blk96 = vw(wfr, 3, 96)
            ones64 = vw(wfr, 1, 64)
            c = [0]
            gb1 = vw(wf2, 128, 1)
            gb2 = vw(wf2, 128, 1)
            gb3 = vw(wf2, 64, 1)
            b2c2 = vw(wf2, 128, 1)
            b3c2 = vw(wf2, 128, 1)
            rb96 = vw(wf2, 96, 1)
            ob1 = vw(wf2, 128, 1)
            ob2 = vw(wf2, 64, 1)
            ones128 = vw(wf2, 1, 128)
            id64 = vw(wf2, 64, 64)

            # zero the scatter accumulator in one DMA
            zbig = cp.tile([128, B + 1, SLOTC], BF16, tag="zbig")
            nc.vector.memset(zbig[:, :, :], 0.0)
            zap = bass.AP(aggin_d[:, :].tensor, 0,
                          [[SLOTC, 128], [128 * SLOTC, B + 1], [1, SLOTC]])
            nc.sync.dma_start(zap, zbig[:, :, :])

            with tc.tile_pool(name="keep", bufs=1) as kp:
                u2s = kp.tile([128, W], F32, tag="u2s")
                v2 = kp.tile([128, EP], F32, tag="v2")
                gg2 = kp.tile([128, W], F32, tag="gg2")
                slotAll = kp.tile([128, T2, SLOTC], F32, tag="slotAll")
                nc.vector.memset(slotAll[:, :, :], 0.0)

                with (
                    tc.tile_pool(name="pre", bufs=1) as pp,
                    tc.tile_pool(name="pps", bufs=1, space="PSUM") as pps,
                ):
                    # v2 = w1c^T ef + b1 (bias via augmented row)
                    vps = pps.tile([64, NE], F32, tag="pa")
                    nc.tensor.matmul(vps[:], w1cA[:], efA[:], start=True,
                                     stop=True)
                    nc.scalar.activation(v2[0:64, :], vps[:, 0:EP], AF.Copy)
                    nc.scalar.activation(v2[64:128, :], vps[:, EP:NE],
                                         AF.Copy)
                    # u2 = blockdup(w1ab)^T [ej; ek]  (both folds identical)
                    u2p = pps.tile([128, W], F32, tag="pb")
                    for k0, k1 in wch():
                        nc.tensor.matmul(u2p[:, k0:k1], w1abD[:],
                                         ejk[:, k0:k1], start=True, stop=True)
                    nc.scalar.activation(u2s[:], u2p[:], AF.Copy)
                    # rbf features
                    r96p = pps.tile([96, W], F32, tag="pb")
                    for k0, k1 in wch():
                        nc.tensor.matmul(r96p[:, k0:k1], blk96[:],
                                         r3[:, k0:k1], start=True, stop=True)
                    sq96 = pp.tile([96, W], F32, tag="sq96")
                    nc.scalar.activation(sq96[:], r96p[:], AF.Square,
                                         bias=rb96[:], scale=RBF_SCALE)
                    grbf = pp.tile([97, W], BF16, tag="grbf")
                    nc.scalar.activation(grbf[0:96, :], sq96[:], AF.Exp,
                                         scale=-0.5)
                    nc.vector.tensor_copy(grbf[96:97, :], cos1[:])
                    # geom MLP
                    gp = pps.tile([128, W], F32, tag="pa")
                    for k0, k1 in wch():
                        nc.tensor.matmul(gp[:, k0:k1], gw1a[:], hjA[:, k0:k1],
                                         start=True, stop=False)
                        nc.tensor.matmul(gp[:, k0:k1], gw1b[:], hkA[:, k0:k1],
                                         start=False, stop=False)
                        nc.tensor.matmul(gp[:, k0:k1], gw1c[:],
                                         grbf[:, k0:k1], start=False,
                                         stop=True)
                    h1g = pp.tile([128, W], BF16, tag="h1g")
                    nc.scalar.activation(h1g[:], gp[:], AF.Silu, bias=gb1[:])
                    gp2 = pps.tile([128, W], F32, tag="pb")
                    for k0, k1 in wch():
                        nc.tensor.matmul(gp2[:, k0:k1], gw2[:], h1g[:, k0:k1],
                                         start=True, stop=True)
                    h2g = pp.tile([128, W], BF16, tag="h2g")
                    nc.scalar.activation(h2g[:], gp2[:], AF.Silu, bias=gb2[:])
                    g3p = pps.tile([64, W], F32, tag="pa")
                    for k0, k1 in wch():
                        nc.tensor.matmul(g3p[:, k0:k1], gw3[:], h2g[:, k0:k1],
                                         start=True, stop=True)
                    # gg2 = cw * (g3 + gb3), duplicated on both folds
                    cwB = pps.tile([64, W], F32, tag="pc")
                    for k0, k1 in wch():
                        nc.tensor.matmul(cwB[:, k0:k1], ones64[:],
                                         cw1[:, k0:k1], start=True, stop=True)
                    t3 = pp.tile([64, W], F32, tag="t3")
                    nc.vector.tensor_scalar_add(t3[:], g3p[:], gb3[:])
                    nc.vector.tensor_tensor(gg2[0:64, :], t3[:], cwB[:],
                                            op=ALU.mult)
                    nc.vector.tensor_copy(gg2[64:128, :], gg2[0:64, :])
                    # Sgg and norm slot columns
                    gg2v = gg2[:, :].rearrange("p (t f) -> p t f", t=T2)
                    nc.vector.tensor_reduce(slotAll[:, :, 40:41], gg2v,
                                            axis=mybir.AxisListType.X,
                                            op=ALU.add)
                    cw1v = cw1[:, :].rearrange("p (t f) -> p t f", t=T2)
                    nc.vector.tensor_reduce(slotAll[0:1, :, 41:42], cw1v,
                                            axis=mybir.AxisListType.X,
                                            op=ALU.add)

                # ---- e-pair loop, software pipelined
                with (
                    tc.tile_pool(name="ph1", bufs=3) as ph1,
                    tc.tile_pool(name="ph2", bufs=3) as ph2,
                    tc.tile_pool(name="pco", bufs=2) as pco,
                    tc.tile_pool(name="psL", bufs=2, space="PSUM") as psL,
                ):
                    h1_t = [None] * EP
                    l2_t = [None] * EP

                    def post_h1(e):
                        t = ph1.tile([128, W], BF16, tag="h1")
                        nc.scalar.activation(t[:], u2s[:], AF.Silu,
                                             bias=v2[:, e:e + 1])
                        h1_t[e] = t

                    def post_l2(e):
                        t = psL.tile([128, W], F32, tag="l2")
                        for k0, k1 in wch():
                            nc.tensor.matmul(t[:, k0:k1], w2bd[:],
                                             h1_t[e][:, k0:k1], start=True,
                                             stop=True)
                        h1_t[e] = None
                        l2_t[e] = t

                    post_h1(0)
                    post_l2(0)
                    for e in range(EP):
                        if e + 1 < EP:
                            post_h1(e + 1)
                        h2 = ph2.tile([128, W], BF16, tag="h2")
                        nc.scalar.activation(h2[:], l2_t[e][:], AF.Silu,
                                             bias=b2c2[:])
                        l2_t[e] = None
                        if e + 1 < EP:
                            post_l2(e + 1)
                        l3 = psL.tile([128, W], F32, tag="l3")
                        for k0, k1 in wch():
                            nc.tensor.matmul(l3[:, k0:k1], w3bd[:],
                                             h2[:, k0:k1], start=True,
                                             stop=True)
                        co = pco.tile([128, W], F32, tag="co")
                        nc.vector.tensor_tensor(co[:], l3[:], gg2[:],
                                                op=ALU.mult)
                        cov = co[:, :].rearrange("p (t f) -> p t f", t=T2)
                        nc.vector.tensor_reduce(slotAll[:, :, e:e + 1], cov,
                                                axis=mybir.AxisListType.X,
                                                op=ALU.add)

                # ---- fold b3*Sgg into the agg columns, cast to bf16
                sggb = kp.tile([128, T2], F32, tag="sggb")
                nc.vector.tensor_scalar(sggb[:, :], slotAll[:, :, 40],
                                        b3c2[:], None, op0=ALU.mult)
                nc.vector.tensor_tensor(slotAll[:, :, 0:40],
                                        slotAll[:, :, 0:40],
                                        _bc_last(sggb[:, :], 40), op=ALU.add)
                slotBf = kp.tile([128, T2, SLOTC], BF16, tag="slotBf")
                nc.vector.tensor_copy(slotBf[:, :, :], slotAll[:, :, :])
                if use_sum:
                    slotS = kp.tile([128, T, SLOTC], BF16, tag="slotS")
                    nc.vector.tensor_tensor(slotS[:, :, :],
                                            slotAll[:, 0::2, :],
                                            slotAll[:, 1::2, :], op=ALU.add)
                for _t in range(T2):
                    nc.gpsimd.indirect_dma_start(
                        out=aggin_d[:, :],
                        out_offset=bass.IndirectOffsetOnAxis(
                            ap=sx[:, _t:_t + 1], axis=0),
                        in_=slotBf[:, _t, :],
                        in_offset=None,
                    )
                if use_sum:
                    for _t in range(T):
                        nc.gpsimd.indirect_dma_start(
                            out=aggin_d[:, :],
                            out_offset=bass.IndirectOffsetOnAxis(
                                ap=sxs[:, _t:_t + 1], axis=0),
                            in_=slotS[:, _t, :],
                            in_offset=None,
                        )
                nc.gpsimd.collective_compute(
                    "ReduceScatter",
                    mybir.AluOpType.add,
                    replica_groups=[list(range(NCORES))],
                    ins=[aggin_d[0:B * 128, :]],
                    outs=[aggout_d[:, :]],
                )

            # ---- endgame: normalize + out-MLP on this core's 2 batches
            with (
                tc.tile_pool(name="eg", bufs=1) as eg,
                tc.tile_pool(name="egp", bufs=2, space="PSUM") as egp,
            ):
                agg2 = eg.tile([128, BL, SLOTC], BF16, tag="agg2")
                nc.sync.dma_start(
                    agg2[:, :, :],
                    bass.AP(aggout_d[:, :].tensor, 0,
                            [[SLOTC, 128], [128 * SLOTC, BL], [1, SLOTC]]))
                norm2 = eg.tile([1, BL], F32, tag="norm2")
                nc.vector.tensor_scalar_max(norm2[:], agg2[0:1, :, 41], 1e-8)
                rn = eg.tile([1, BL], F32, tag="rn")
                nc.vector.reciprocal(rn[:], norm2[:])
                rnp = egp.tile([128, BL], F32, tag="rnp")
                nc.tensor.matmul(rnp[:], ones128[:], rn[:], start=True,
                                 stop=True)
                agn = eg.tile([128, BL, EP], F32, tag="agn")
                nc.vector.tensor_tensor(agn[:], agg2[:, :, 0:40],
                                        _bc_last(rnp[:, :], EP), op=ALU.mult)
                unf = eg.tile([64, BL, NE], F32R, tag="unf")
                nc.vector.tensor_copy(unf[:, :, 0::2], agn[0:64, :, :])
                nc.vector.tensor_copy(unf[:, :, 1::2], agn[64:128, :, :])

                NCOL = BL * NE
                unf_f = unf[:, :, :].rearrange("p a b -> p (a b)")
                hop = egp.tile([128, NCOL], F32, tag="hop")
                nc.tensor.matmul(hop[:], ow1[:], unf_f[:], start=True,
                                 stop=True)
                ho = eg.tile([128, BL, NE], F32R, tag="ho")
                ho_f = ho[:, :, :].rearrange("p a b -> p (a b)")
                nc.scalar.activation(ho_f[:], hop[:], AF.Silu, bias=ob1[:])
                o2p = egp.tile([64, NCOL], F32, tag="o2p")
                nc.tensor.matmul(o2p[:], ow2[:], ho_f[:], start=True,
                                 stop=True)
                outf = eg.tile([64, BL, NE], F32, tag="outf")
                outf_f = outf[:, :, :].rearrange("p a b -> p (a b)")
                nc.vector.tensor_scalar_add(outf_f[:], o2p[:], ob2[:])
                t80 = eg.tile([NE, BL, S], F32, tag="t80")
                for bl in range(BL):
                    tp = egp.tile([NE, S], F32, tag="tp")
                    nc.tensor.transpose(tp[:], outf[:, bl, :], id64[:])
                    nc.scalar.copy(t80[:, bl, :], tp[:])
                    nc.sync.dma_start(out_d[bl * NE:(bl + 1) * NE, :],
                                      t80[:, bl, :])
    nc.compile()
    return nc


def _get_nc(T, use_sum):
    if (T, use_sum) not in _NC_CACHE:
        _NC_CACHE[(T, use_sum)] = build_nc(T, use_sum)
    return _NC_CACHE[(T, use_sum)]


def _cutoff(r):
    return np.where(r < CUT,
                    0.5 * (np.cos(np.pi * np.minimum(r, CUT) / CUT) + 1.0),
                    0.0).astype(np.float32)


def _prep(inputs):
    h = np.asarray(inputs["h_flat"], dtype=np.float32)
    z = np.asarray(inputs["z_flat"]).astype(np.int64)
    ef = np.asarray(inputs["e_feat"], dtype=np.float32)
    pj = np.asarray(inputs["path_j"]).astype(np.int64)
    pk = np.asarray(inputs["path_k"]).astype(np.int64)
    r0j = np.asarray(inputs["path_r0j"], dtype=np.float32)
    r0k = np.asarray(inputs["path_r0k"], dtype=np.float32)
    rjk = np.asarray(inputs["path_rjk"], dtype=np.float32)
    cosa = np.asarray(inputs["path_cosangle"], dtype=np.float32)
    pb = np.asarray(inputs["path_batch"]).astype(np.int64)
    zemb = np.asarray(inputs["z_emb"], dtype=np.float32)
    assert int(inputs["bsz"]) == B

    cw = _cutoff(r0j) * _cutoff(r0k) * _cutoff(rjk)
    keep = (r0j < CUT) & (r0k < CUT) & (rjk < CUT)
    order = np.argsort(pb, kind="stable")
    order = order[keep[order]]

    # half-tiles of <= 64 paths, per batch
    halves = []
    for b in range(B):
        idxs = order[pb[order] == b]
        for k0 in range(0, len(idxs), HF):
            halves.append((b, idxs[k0:k0 + HF]))

    # distribute: at most one half-tile per (core, batch) when possible
    core_halves = [[] for _ in range(NCORES)]
    core_batches = [dict() for _ in range(NCORES)]  # batch -> count
    rr = 0
    overflow = []
    for (b, idxs) in halves:
        placed = False
        for k in range(NCORES):
            c = (rr + k) % NCORES
            if b not in core_batches[c]:
                core_halves[c].append((b, idxs))
                core_batches[c][b] = 1
                rr = (c + 1) % NCORES
                placed = True
                break
        if not placed:
            overflow.append((b, idxs))
    use_sum = len(overflow) > 0
    merged = [dict() for _ in range(NCORES)]  # core -> {tile_idx: batch}
    if use_sum:
        # place each overflow half next to its same-batch sibling as the
        # two halves of one tile; that tile scatters only its summed block
        for (b, idxs) in overflow:
            done = False
            for c in range(NCORES):
                if core_batches[c].get(b, 0) == 1:
                    j = next(i for i, (bb, _) in enumerate(core_halves[c])
                             if bb == b)
                    lst = core_halves[c]
                    # move sibling to an even index at the end, pair them
                    sib = lst.pop(j)
                    if len(lst) % 2 == 1:
                        lst.append((None, np.empty(0, np.int64)))
                    ti = len(lst) // 2
                    lst.append(sib)
                    lst.append((b, idxs))
                    core_batches[c][b] = 2
                    merged[c][ti] = b
                    done = True
                    break
            assert done, "batch needs >2 half-tiles on one core"

    T = max(1, max((len(ch) + 1) // 2 for ch in core_halves))
    T2 = 2 * T
    W = T2 * HF
    hT = h.T  # (128, 1024)
    ezT = zemb.T  # (32, 101)

    in_maps = []
    for c in range(NCORES):
        ch = list(core_halves[c])
        while len(ch) < T2:
            ch.append((None, np.empty(0, np.int64)))
        hjT = np.zeros((ATOM, W), np.float32)
        hkT = np.zeros((ATOM, W), np.float32)
        ejk = np.zeros((64, W), np.float32)
        r3 = np.full((3, W), CUT, np.float32)
        cos1 = np.zeros((1, W), np.float32)
        cw1 = np.zeros((1, W), np.float32)
        sx = np.empty((128, T2), np.int32)
        sxs = np.full((128, T), B * 128, np.int32)
        sxs += np.arange(128, dtype=np.int32)[:, None]
        for j, (b, idxs) in enumerate(ch):
            ti = j // 2
            if merged[c].get(ti) is not None and b is not None:
                # merged tile: halves scatter to trash, sum goes to batch
                sx[:, j] = B * 128 + np.arange(128)
                if j % 2 == 0:
                    sxs[:, ti] = b * 128 + np.arange(128)
            elif b is None:
                sx[:, j] = B * 128 + np.arange(128)
            else:
                sx[:, j] = b * 128 + np.arange(128)
            n = len(idxs)
            if n == 0:
                continue
            cols = slice(j * HF, j * HF + n)
            hjT[:, cols] = hT[:, pj[idxs]]
            hkT[:, cols] = hT[:, pk[idxs]]
            ejk[0:32, cols] = ezT[:, z[pj[idxs]]]
            ejk[32:64, cols] = ezT[:, z[pk[idxs]]]
            r3[0, cols] = np.minimum(r0j[idxs], CUT)
            r3[1, cols] = np.minimum(r0k[idxs], CUT)
            r3[2, cols] = np.minimum(rjk[idxs], CUT)
            cos1[0, cols] = cosa[idxs]
            cw1[0, cols] = cw[idxs]
        in_maps.append({
            "hjT": hjT.astype(NPBF16), "hkT": hkT.astype(NPBF16),
            "ejk": ejk.astype(NPBF16), "r3": r3, "cos1": cos1, "cw1": cw1,
            "sx": sx, "sxs": sxs,
        })

    # ---- replicated params
    gm_w1 = np.asarray(inputs["gm_w1"], np.float32)
    pe_w1 = np.asarray(inputs["pe_w1"], np.float32)
    pe_w2 = np.asarray(inputs["pe_w2"], np.float32)
    pe_w3 = np.asarray(inputs["pe_w3"], np.float32)
    pe_b1 = np.asarray(inputs["pe_b1"], np.float32)
    pe_b2 = np.asarray(inputs["pe_b2"], np.float32)
    pe_b3 = np.asarray(inputs["pe_b3"], np.float32)
    w1ab = pe_w1[0:64, :]
    w1abD = np.concatenate([w1ab, w1ab], axis=1)  # [64, 128]
    w1cA = np.concatenate([pe_w1[64:96, :], pe_b1[None, :]], axis=0)  # [33,64]
    efT = ef.T  # [32, 80]
    efA = np.concatenate(
        [np.concatenate([efT[:, 0::2], efT[:, 1::2]], axis=1),
         np.ones((1, NE), np.float32)], axis=0)  # [33, 80]
    w2bd = np.zeros((128, 128), np.float32)
    w2bd[0:64, 0:64] = pe_w2
    w2bd[64:128, 64:128] = pe_w2
    w3bd = np.zeros((128, 128), np.float32)
    w3bd[0:64, 0:64] = pe_w3
    w3bd[64:128, 64:128] = pe_w3
    blk96 = np.zeros((3, 96), np.float32)
    for k in range(3):
        blk96[k, 32 * k:32 * (k + 1)] = 1.0
    wbf = np.zeros((128, 1104), np.float32)
    cc = [0]

    def put(arr, t):
        r, k = arr.shape
        t[0:r, cc[0]:cc[0] + k] = arr
        cc[0] += k
    put(gm_w1[0:128, :], wbf)
    put(gm_w1[128:256, :], wbf)
    put(gm_w1[256:353, :], wbf)
    put(np.asarray(inputs["gm_w2"], np.float32), wbf)
    put(np.asarray(inputs["gm_w3"], np.float32), wbf)
    put(w2bd, wbf)
    put(w3bd, wbf)
    put(w1abD, wbf)
    put(w1cA, wbf)
    put(efA, wbf)
    wfr = np.zeros((128, 352), np.float32)
    cc = [0]
    put(np.asarray(inputs["op_w1"], np.float32), wfr)
    put(np.asarray(inputs["op_w2"], np.float32), wfr)
    put(blk96, wfr)
    put(np.ones((1, 64), np.float32), wfr)
    wf2 = np.zeros((128, 208), np.float32)
    cc = [0]
    put(np.asarray(inputs["gm_b1"], np.float32)[:, None], wf2)
    put(np.asarray(inputs["gm_b2"], np.float32)[:, None], wf2)
    put(np.asarray(inputs["gm_b3"], np.float32)[:, None], wf2)
    put(np.concatenate([pe_b2, pe_b2])[:, None].astype(np.float32), wf2)
    put(np.concatenate([pe_b3, pe_b3])[:, None].astype(np.float32), wf2)
    put(-np.tile(np.arange(RBF, dtype=np.float32), 3)[:, None], wf2)
    put(np.asarray(inputs["op_b1"], np.float32)[:, None], wf2)
    put(np.asarray(inputs["op_b2"], np.float32)[:, None], wf2)
    put(np.ones((1, 128), np.float32), wf2)
    put(np.eye(64, dtype=np.float32), wf2)
    params = {
        "wbf": wbf.astype(NPBF16),
        "wfr": wfr,
        "wf2": wf2,
    }
    for m in in_maps:
        m.update(params)
    return T, use_sum, in_maps


def _ensure_ntff_hook():
    """Inject antenv.axon_hooks (missing in this image) so trace=True works."""
    try:
        from antenv.axon_hooks import get_axon_ntff_profile_hook  # noqa: F401
        return
    except ImportError:
        pass
    import sys
    import types

    import antenv
    mod = types.ModuleType("antenv.axon_hooks")
    mod._hook = None
    mod.set_axon_ntff_profile_hook = lambda h: setattr(mod, "_hook", h)
    mod.get_axon_ntff_profile_hook = lambda: mod._hook
    sys.modules["antenv.axon_hooks"] = mod
    antenv.axon_hooks = mod
    try:
        from trn_agent_boot.trn_boot import _ntff_profile_via_ctypes
        mod._hook = _ntff_profile_via_ctypes("/opt/axon/libaxon_pjrt.so")
    except Exception as e:  # degrade to no-trace
        print("ntff hook setup failed:", e)


def kernel(**inputs) -> np.ndarray:
    T, use_sum, in_maps = _prep(inputs)
    nc = _get_nc(T, use_sum)
    trace = bool(int(os.environ.get("KERNEL_TRACE", "0")))
    if trace:
        _ensure_ntff_hook()
        import concourse.bass_utils as _bu
        _bu.upload_artifacts = lambda d: "local"
    res = run_bass_kernel_spmd(nc, in_maps, list(range(NCORES)), trace=trace,
                               tmpdir=os.environ.get("KERNEL_TRACE_DIR"))
    global LAST_RESULTS
    LAST_RESULTS = res
    out = np.empty((B, NE, S), np.float32)
    for c in range(NCORES):
        oc = np.asarray(res.results[c]["out"], np.float32).reshape(BL, NE, S)
        for bl in range(BL):
            out[BL * c + bl] = oc[bl]
    return out


LAST_RESULTS = None


# revision 12
# speedup vs baseline: 1.0547x; 1.0547x over previous
"""Trainium2 Bass kernel for nn_AbsorberPathAggregator (v2).

Strategy: host-side path filtering (cutoff weight == 0 exactly for ~42% of
paths -> zero contribution, dropped exactly), survivors packed into 64-path
half-tiles batch-padded to 64, half-tiles distributed so each (core, batch)
owns at most one (seed-stable; a sum-merge variant handles up to two).

Device pipeline per core (W = T*128 path columns, T ~ 5):
  prepass: u = w1ab^T [ej;ek] (one matmul), v(e)+b1 (one matmul), rbf via
    broadcast-matmul + Square/Exp, geom MLP in bf16, gg2 = cw*(g3+b3) on DVE.
  e-loop over 40 folded e-pairs, software-pipelined across engines:
    scalar : h1 = silu(u (+) v_e)   -- the add rides the ACTIVATE bias port
    tensor : L2p = w2bd @ h1        (bf16, 640-wide moving)
    scalar : h2 = silu(L2p + b2)
    tensor : L3p = w3bd @ h2
    vector : contrib = L3p * gg2 ; slot cols = reduce_f(contrib) per half-tile
  tail: indirect-scatter slot blocks into batch-indexed DRAM accumulator,
    one ReduceScatter (each core receives exactly its 2 batches), then the
    out-projection on 2 batches and a direct store.
"""

import os

import numpy as np
import ml_dtypes

import concourse.bacc as bacc
import concourse.bass as bass
import concourse.mybir as mybir
import concourse.tile as tile
from concourse.bass_utils import run_bass_kernel_spmd

F32 = mybir.dt.float32
F32R = mybir.dt.float32r
BF16 = mybir.dt.bfloat16
I32 = mybir.dt.int32
NPBF16 = ml_dtypes.bfloat16

NCORES = 8
B = 16
BL = 2              # batches per core after ReduceScatter
NE = 80
S = 64
EP = NE // 2        # folded e-pairs
HF = 64             # paths per half-tile
SLOTC = 42          # 40 agg cols + Sgg col + norm col
ATOM = 128
RBF = 32
CUT = 5.0
RBF_SCALE = (RBF - 1) / CUT

_NC_CACHE = {}


def _bc_last(ap, n):
    """[...dims] -> [...dims, n] with 0-step last dim."""
    l = [list(x) for x in ap.ap]
    return bass.AP(ap.tensor, ap.offset, l + [[0, n]])


def build_nc(T: int, use_sum: bool) -> bass.Bass:
    nc = bacc.Bacc("TRN2", target_bir_lowering=False, debug=False,
                   num_devices=NCORES)
    AF = mybir.ActivationFunctionType
    ALU = mybir.AluOpType
    T2 = 2 * T
    W = T2 * HF

    def wch():
        out, k = [], 0
        while k < W:
            out.append((k, min(k + 512, W)))
            k += 512
        return out

    # ---- per-core inputs
    hjT_d = nc.dram_tensor("hjT", [ATOM, W], BF16, kind="ExternalInput")
    hkT_d = nc.dram_tensor("hkT", [ATOM, W], BF16, kind="ExternalInput")
    ejk_d = nc.dram_tensor("ejk", [64, W], BF16, kind="ExternalInput")
    r3_d = nc.dram_tensor("r3", [3, W], F32R, kind="ExternalInput")
    cos1_d = nc.dram_tensor("cos1", [1, W], F32, kind="ExternalInput")
    cw1_d = nc.dram_tensor("cw1", [1, W], F32R, kind="ExternalInput")
    sx_d = nc.dram_tensor("sx", [128, T2], I32, kind="ExternalInput")
    sxs_d = nc.dram_tensor("sxs", [128, T], I32, kind="ExternalInput")
    gidx_d = nc.dram_tensor("gidx", [128, BL], I32, kind="ExternalInput")
    # ---- packed replicated params (one DMA each)
    wbf_d = nc.dram_tensor("wbf", [128, 1104], BF16, kind="ExternalInput")
    wfr_d = nc.dram_tensor("wfr", [128, 352], F32R, kind="ExternalInput")
    wf2_d = nc.dram_tensor("wf2", [128, 208], F32, kind="ExternalInput")
    # ---- output + collective buffers
    out_d = nc.dram_tensor("out", [BL * NE, S], F32, kind="ExternalOutput")
    aggin_d = nc.dram_tensor("aggin", [(B + 1) * 128, SLOTC], BF16)
    aggout_d = nc.dram_tensor("aggout", [B * 128, SLOTC], BF16,
                              addr_space="Shared")

    with tile.TileContext(nc) as tc:
        with tc.tile_pool(name="const", bufs=1) as cp:
            def cl(dram, shape, dt, eng):
                t = cp.tile(shape, dt, tag=dram.name)
                eng.dma_start(t[:], dram[:])
                return t

            # ACT-table warmup: pull the Exp set in before anything needs it
            warm = cp.tile([1, 8], F32, tag="warm")
            nc.vector.memset(warm[:], 0.25)
            nc.scalar.activation(warm[0:1, 0:1], warm[0:1, 1:2], AF.Exp)

            # critical-path loads on the sync HWDGE ring
            ejk = cl(ejk_d, [64, W], BF16, nc.sync)
            wbf = cl(wbf_d, [128, 1104], BF16, nc.sync)
            r3 = cl(r3_d, [3, W], F32R, nc.sync)
            cw1 = cl(cw1_d, [1, W], F32R, nc.sync)
            hjA = cl(hjT_d, [ATOM, W], BF16, nc.sync)
            hkA = cl(hkT_d, [ATOM, W], BF16, nc.sync)
            # the rest via gpsimd SWDGE (its queue is idle here)
            wfr = cl(wfr_d, [128, 352], F32R, nc.gpsimd)
            wf2 = cl(wf2_d, [128, 208], F32, nc.gpsimd)
            cos1 = cl(cos1_d, [1, W], F32, nc.gpsimd)
            sx = cl(sx_d, [128, T2], I32, nc.gpsimd)
            gx = cl(gidx_d, [128, BL], I32, nc.gpsimd)
            sxs = cl(sxs_d, [128, T], I32, nc.gpsimd) if use_sum else None

            # views into the packed param tiles
            c = [0]

            def vw(t, rows, cols):
                a = t[0:rows, c[0]:c[0] + cols]
                c[0] += cols
                return a
            gw1a = vw(wbf, 128, 128)
            gw1b = vw(wbf, 128, 128)
            gw1c = vw(wbf, 97, 128)
            gw2 = vw(wbf, 128, 128)
            gw3 = vw(wbf, 128, 64)
            w2bd = vw(wbf, 128, 128)
            w3bd = vw(wbf, 128, 128)
            w1abD = vw(wbf, 64, 128)
            w1cA = vw(wbf, 33, 64)
            efA = vw(wbf, 33, NE)
            c = [0]
            ow1 = vw(wfr, 64, 128)
            ow2 = vw(wfr, 128, 64)
            blk96 = vw(wfr, 3, 96)
            ones64 = vw(wfr, 1, 64)
            c = [0]
            gb1 = vw(wf2, 128, 1)
            gb2 = vw(wf2, 128, 1)
            gb3 = vw(wf2, 64, 1)
            b2c2 = vw(wf2, 128, 1)
            b3c2 = vw(wf2, 128, 1)
            rb96 = vw(wf2, 96, 1)
            ob1 = vw(wf2, 128, 1)
            ob2 = vw(wf2, 64, 1)
            ones128 = vw(wf2, 1, 128)
            id64 = vw(wf2, 64, 64)

            # zero the scatter accumulator in one DMA
            zbig = cp.tile([128, B + 1, SLOTC], BF16, tag="zbig")
            nc.vector.memset(zbig[:, :, :], 0.0)
            zap = bass.AP(aggin_d[:, :].tensor, 0,
                          [[SLOTC, 128], [128 * SLOTC, B + 1], [1, SLOTC]])
            nc.sync.dma_start(zap, zbig[:, :, :])

            with tc.tile_pool(name="keep", bufs=1) as kp:
                u2s = kp.tile([128, W], F32, tag="u2s")
                v2 = kp.tile([128, EP], F32, tag="v2")
                gg2 = kp.tile([128, W], F32, tag="gg2")
                slotAll = kp.tile([128, T2, SLOTC], F32, tag="slotAll")
                nc.vector.memset(slotAll[:, :, :], 0.0)

                with (
                    tc.tile_pool(name="pre", bufs=1) as pp,
                    tc.tile_pool(name="pps", bufs=1, space="PSUM") as pps,
                ):
                    # v2 = w1c^T ef + b1 (bias via augmented row)
                    vps = pps.tile([64, NE], F32, tag="pa")
                    nc.tensor.matmul(vps[:], w1cA[:], efA[:], start=True,
                                     stop=True)
                    nc.scalar.activation(v2[0:64, :], vps[:, 0:EP], AF.Copy)
                    nc.scalar.activation(v2[64:128, :], vps[:, EP:NE],
                                         AF.Copy)
                    # u2 = blockdup(w1ab)^T [ej; ek]  (both folds identical)
                    u2p = pps.tile([128, W], F32, tag="pb")
                    for k0, k1 in wch():
                        nc.tensor.matmul(u2p[:, k0:k1], w1abD[:],
                                         ejk[:, k0:k1], start=True, stop=True)
                    nc.scalar.activation(u2s[:], u2p[:], AF.Copy)
                    # rbf features
                    r96p = pps.tile([96, W], F32, tag="pb")
                    for k0, k1 in wch():
                        nc.tensor.matmul(r96p[:, k0:k1], blk96[:],
                                         r3[:, k0:k1], start=True, stop=True)
                    sq96 = pp.tile([96, W], F32, tag="sq96")
                    nc.scalar.activation(sq96[:], r96p[:], AF.Square,
                                         bias=rb96[:], scale=RBF_SCALE)
                    grbf = pp.tile([97, W], BF16, tag="grbf")
                    nc.scalar.activation(grbf[0:96, :], sq96[:], AF.Exp,
                                         scale=-0.5)
                    nc.vector.tensor_copy(grbf[96:97, :], cos1[:])
                    # geom MLP
                    gp = pps.tile([128, W], F32, tag="pa")
                    for k0, k1 in wch():
                        nc.tensor.matmul(gp[:, k0:k1], gw1a[:], hjA[:, k0:k1],
                                         start=True, stop=False)
                        nc.tensor.matmul(gp[:, k0:k1], gw1b[:], hkA[:, k0:k1],
                                         start=False, stop=False)
                        nc.tensor.matmul(gp[:, k0:k1], gw1c[:],
                                         grbf[:, k0:k1], start=False,
                                         stop=True)
                    h1g = pp.tile([128, W], BF16, tag="h1g")
                    nc.scalar.activation(h1g[:], gp[:], AF.Silu, bias=gb1[:])
                    gp2 = pps.tile([128, W], F32, tag="pb")
                    for k0, k1 in wch():
                        nc.tensor.matmul(gp2[:, k0:k1], gw2[:], h1g[:, k0:k1],
                                         start=True, stop=True)
                    h2g = pp.tile([128, W], BF16, tag="h2g")
                    nc.scalar.activation(h2g[:], gp2[:], AF.Silu, bias=gb2[:])
                    g3p = pps.tile([64, W], F32, tag="pa")
                    for k0, k1 in wch():
                        nc.tensor.matmul(g3p[:, k0:k1], gw3[:], h2g[:, k0:k1],
                                         start=True, stop=True)
                    # gg2 = cw * (g3 + gb3), duplicated on both folds
                    cwB = pps.tile([64, W], F32, tag="pc")
                    for k0, k1 in wch():
                        nc.tensor.matmul(cwB[:, k0:k1], ones64[:],
                                         cw1[:, k0:k1], start=True, stop=True)
                    t3 = pp.tile([64, W], F32, tag="t3")
                    nc.vector.tensor_scalar_add(t3[:], g3p[:], gb3[:])
                    nc.vector.tensor_tensor(gg2[0:64, :], t3[:], cwB[:],
                                            op=ALU.mult)
                    nc.vector.tensor_copy(gg2[64:128, :], gg2[0:64, :])
                    # Sgg and norm slot columns
                    gg2v = gg2[:, :].rearrange("p (t f) -> p t f", t=T2)
                    nc.vector.tensor_reduce(slotAll[:, :, 40:41], gg2v,
                                            axis=mybir.AxisListType.X,
                                            op=ALU.add)
                    cw1v = cw1[:, :].rearrange("p (t f) -> p t f", t=T2)
                    nc.vector.tensor_reduce(slotAll[0:1, :, 41:42], cw1v,
                                            axis=mybir.AxisListType.X,
                                            op=ALU.add)

                # ---- e-pair loop, software pipelined
                with (
                    tc.tile_pool(name="ph1", bufs=3) as ph1,
                    tc.tile_pool(name="ph2", bufs=3) as ph2,
                    tc.tile_pool(name="pco", bufs=2) as pco,
                    tc.tile_pool(name="psL", bufs=2, space="PSUM") as psL,
                ):
                    h1_t = [None] * EP
                    l2_t = [None] * EP

                    def post_h1(e):
                        t = ph1.tile([128, W], BF16, tag="h1")
                        nc.scalar.activation(t[:], u2s[:], AF.Silu,
                                             bias=v2[:, e:e + 1])
                        h1_t[e] = t

                    def post_l2(e):
                        t = psL.tile([128, W], F32, tag="l2")
                        for k0, k1 in wch():
                            nc.tensor.matmul(t[:, k0:k1], w2bd[:],
                                             h1_t[e][:, k0:k1], start=True,
                                             stop=True)
                        h1_t[e] = None
                        l2_t[e] = t

                    post_h1(0)
                    post_l2(0)
                    for e in range(EP):
                        if e + 1 < EP:
                            post_h1(e + 1)
                        h2 = ph2.tile([128, W], BF16, tag="h2")
                        nc.scalar.activation(h2[:], l2_t[e][:], AF.Silu,
                                             bias=b2c2[:])
                        l2_t[e] = None
                        if e + 1 < EP:
                            post_l2(e + 1)
                        l3 = psL.tile([128, W], F32, tag="l3")
                        for k0, k1 in wch():
                            nc.tensor.matmul(l3[:, k0:k1], w3bd[:],
                                             h2[:, k0:k1], start=True,
                                             stop=True)
                        co = pco.tile([128, W], F32, tag="co")
                        nc.vector.tensor_tensor(co[:], l3[:], gg2[:],
                                                op=ALU.mult)
                        cov = co[:, :].rearrange("p (t f) -> p t f", t=T2)
                        nc.vector.tensor_reduce(slotAll[:, :, e:e + 1], cov,
                                                axis=mybir.AxisListType.X,
                                                op=ALU.add)

                # ---- fold b3*Sgg into the agg columns, cast to bf16
                sggb = kp.tile([128, T2], F32, tag="sggb")
                nc.vector.tensor_scalar(sggb[:, :], slotAll[:, :, 40],
                                        b3c2[:], None, op0=ALU.mult)
                nc.vector.tensor_tensor(slotAll[:, :, 0:40],
                                        slotAll[:, :, 0:40],
                                        _bc_last(sggb[:, :], 40), op=ALU.add)
                slotBf = kp.tile([128, T2, SLOTC], BF16, tag="slotBf")
                nc.vector.tensor_copy(slotBf[:, :, :], slotAll[:, :, :])
                if use_sum:
                    slotS = kp.tile([128, T, SLOTC], BF16, tag="slotS")
                    nc.vector.tensor_tensor(slotS[:, :, :],
                                            slotAll[:, 0::2, :],
                                            slotAll[:, 1::2, :], op=ALU.add)
                for _t in range(T2):
                    nc.gpsimd.indirect_dma_start(
                        out=aggin_d[:, :],
                        out_offset=bass.IndirectOffsetOnAxis(
                            ap=sx[:, _t:_t + 1], axis=0),
                        in_=slotBf[:, _t, :],
                        in_offset=None,
                    )
                if use_sum:
                    for _t in range(T):
                        nc.gpsimd.indirect_dma_start(
                            out=aggin_d[:, :],
                            out_offset=bass.IndirectOffsetOnAxis(
                                ap=sxs[:, _t:_t + 1], axis=0),
                            in_=slotS[:, _t, :],
                            in_offset=None,
                        )
                nc.gpsimd.collective_compute(
                    "AllReduce",
                    mybir.AluOpType.add,
                    replica_groups=[list(range(NCORES))],
                    ins=[aggin_d[0:B * 128, :]],
                    outs=[aggout_d[:, :]],
                )

            # ---- endgame: normalize + out-MLP on this core's 2 batches
            with (
                tc.tile_pool(name="eg", bufs=1) as eg,
                tc.tile_pool(name="egp", bufs=2, space="PSUM") as egp,
            ):
                agg2 = eg.tile([128, BL, SLOTC], BF16, tag="agg2")
                for bl in range(BL):
                    nc.gpsimd.indirect_dma_start(
                        out=agg2[:, bl, :],
                        out_offset=None,
                        in_=aggout_d[:, :],
                        in_offset=bass.IndirectOffsetOnAxis(
                            ap=gx[:, bl:bl + 1], axis=0),
                    )
                norm2 = eg.tile([1, BL], F32, tag="norm2")
                nc.vector.tensor_scalar_max(norm2[:], agg2[0:1, :, 41], 1e-8)
                rn = eg.tile([1, BL], F32, tag="rn")
                nc.vector.reciprocal(rn[:], norm2[:])
                rnp = egp.tile([128, BL], F32, tag="rnp")
                nc.tensor.matmul(rnp[:], ones128[:], rn[:], start=True,
                                 stop=True)
                agn = eg.tile([128, BL, EP], F32, tag="agn")
                nc.vector.tensor_tensor(agn[:], agg2[:, :, 0:40],
                                        _bc_last(rnp[:, :], EP), op=ALU.mult)
                unf = eg.tile([64, BL, NE], F32R, tag="unf")
                nc.vector.tensor_copy(unf[:, :, 0::2], agn[0:64, :, :])
                nc.vector.tensor_copy(unf[:, :, 1::2], agn[64:128, :, :])

                NCOL = BL * NE
                unf_f = unf[:, :, :].rearrange("p a b -> p (a b)")
                hop = egp.tile([128, NCOL], F32, tag="hop")
                nc.tensor.matmul(hop[:], ow1[:], unf_f[:], start=True,
                                 stop=True)
                ho = eg.tile([128, BL, NE], F32R, tag="ho")
                ho_f = ho[:, :, :].rearrange("p a b -> p (a b)")
                nc.scalar.activation(ho_f[:], hop[:], AF.Silu, bias=ob1[:])
                o2p = egp.tile([64, NCOL], F32, tag="o2p")
                nc.tensor.matmul(o2p[:], ow2[:], ho_f[:], start=True,
                                 stop=True)
                outf = eg.tile([64, BL, NE], F32, tag="outf")
                outf_f = outf[:, :, :].rearrange("p a b -> p (a b)")
                nc.vector.tensor_scalar_add(outf_f[:], o2p[:], ob2[:])
                t80 = eg.tile([NE, BL, S], F32, tag="t80")
                for bl in range(BL):
                    tp = egp.tile([NE, S], F32, tag="tp")
                    nc.tensor.transpose(tp[:], outf[:, bl, :], id64[:])
                    nc.scalar.copy(t80[:, bl, :], tp[:])
                    nc.sync.dma_start(out_d[bl * NE:(bl + 1) * NE, :],
                                      t80[:, bl, :])
    nc.compile()
    return nc


def _get_nc(T, use_sum):
    if (T, use_sum) not in _NC_CACHE:
        _NC_CACHE[(T, use_sum)] = build_nc(T, use_sum)
    return _NC_CACHE[(T, use_sum)]


def _cutoff(r):
    return np.where(r < CUT,
                    0.5 * (np.cos(np.pi * np.minimum(r, CUT) / CUT) + 1.0),
                    0.0).astype(np.float32)


def _prep(inputs):
    h = np.asarray(inputs["h_flat"], dtype=np.float32)
    z = np.asarray(inputs["z_flat"]).astype(np.int64)
    ef = np.asarray(inputs["e_feat"], dtype=np.float32)
    pj = np.asarray(inputs["path_j"]).astype(np.int64)
    pk = np.asarray(inputs["path_k"]).astype(np.int64)
    r0j = np.asarray(inputs["path_r0j"], dtype=np.float32)
    r0k = np.asarray(inputs["path_r0k"], dtype=np.float32)
    rjk = np.asarray(inputs["path_rjk"], dtype=np.float32)
    cosa = np.asarray(inputs["path_cosangle"], dtype=np.float32)
    pb = np.asarray(inputs["path_batch"]).astype(np.int64)
    zemb = np.asarray(inputs["z_emb"], dtype=np.float32)
    assert int(inputs["bsz"]) == B

    cw = _cutoff(r0j) * _cutoff(r0k) * _cutoff(rjk)
    keep = (r0j < CUT) & (r0k < CUT) & (rjk < CUT)
    order = np.argsort(pb, kind="stable")
    order = order[keep[order]]

    # half-tiles of <= 64 paths, per batch
    halves = []
    for b in range(B):
        idxs = order[pb[order] == b]
        for k0 in range(0, len(idxs), HF):
            halves.append((b, idxs[k0:k0 + HF]))

    # distribute: at most one half-tile per (core, batch) when possible
    core_halves = [[] for _ in range(NCORES)]
    core_batches = [dict() for _ in range(NCORES)]  # batch -> count
    rr = 0
    overflow = []
    for (b, idxs) in halves:
        placed = False
        for k in range(NCORES):
            c = (rr + k) % NCORES
            if b not in core_batches[c]:
                core_halves[c].append((b, idxs))
                core_batches[c][b] = 1
                rr = (c + 1) % NCORES
                placed = True
                break
        if not placed:
            overflow.append((b, idxs))
    use_sum = len(overflow) > 0
    merged = [dict() for _ in range(NCORES)]  # core -> {tile_idx: batch}
    if use_sum:
        # place each overflow half next to its same-batch sibling as the
        # two halves of one tile; that tile scatters only its summed block
        for (b, idxs) in overflow:
            done = False
            for c in range(NCORES):
                if core_batches[c].get(b, 0) == 1:
                    j = next(i for i, (bb, _) in enumerate(core_halves[c])
                             if bb == b)
                    lst = core_halves[c]
                    # move sibling to an even index at the end, pair them
                    sib = lst.pop(j)
                    if len(lst) % 2 == 1:
                        lst.append((None, np.empty(0, np.int64)))
                    ti = len(lst) // 2
                    lst.append(sib)
                    lst.append((b, idxs))
                    core_batches[c][b] = 2
                    merged[c][ti] = b
                    done = True
                    break
            assert done, "batch needs >2 half-tiles on one core"

    T = max(1, max((len(ch) + 1) // 2 for ch in core_halves))
    T2 = 2 * T
    W = T2 * HF
    hT = h.T  # (128, 1024)
    ezT = zemb.T  # (32, 101)

    in_maps = []
    for c in range(NCORES):
        ch = list(core_halves[c])
        while len(ch) < T2:
            ch.append((None, np.empty(0, np.int64)))
        hjT = np.zeros((ATOM, W), np.float32)
        hkT = np.zeros((ATOM, W), np.float32)
        ejk = np.zeros((64, W), np.float32)
        r3 = np.full((3, W), CUT, np.float32)
        cos1 = np.zeros((1, W), np.float32)
        cw1 = np.zeros((1, W), np.float32)
        sx = np.empty((128, T2), np.int32)
        sxs = np.full((128, T), B * 128, np.int32)
        sxs += np.arange(128, dtype=np.int32)[:, None]
        for j, (b, idxs) in enumerate(ch):
            ti = j // 2
            if merged[c].get(ti) is not None and b is not None:
                # merged tile: halves scatter to trash, sum goes to batch
                sx[:, j] = B * 128 + np.arange(128)
                if j % 2 == 0:
                    sxs[:, ti] = b * 128 + np.arange(128)
            elif b is None:
                sx[:, j] = B * 128 + np.arange(128)
            else:
                sx[:, j] = b * 128 + np.arange(128)
            n = len(idxs)
            if n == 0:
                continue
            cols = slice(j * HF, j * HF + n)
            hjT[:, cols] = hT[:, pj[idxs]]
            hkT[:, cols] = hT[:, pk[idxs]]
            ejk[0:32, cols] = ezT[:, z[pj[idxs]]]
            ejk[32:64, cols] = ezT[:, z[pk[idxs]]]
            r3[0, cols] = np.minimum(r0j[idxs], CUT)
            r3[1, cols] = np.minimum(r0k[idxs], CUT)
            r3[2, cols] = np.minimum(rjk[idxs], CUT)
            cos1[0, cols] = cosa[idxs]
            cw1[0, cols] = cw[idxs]
        gidx = np.empty((128, BL), np.int32)
        for bl in range(BL):
            gidx[:, bl] = (BL * c + bl) * 128 + np.arange(128)
        in_maps.append({
            "hjT": hjT.astype(NPBF16), "hkT": hkT.astype(NPBF16),
            "ejk": ejk.astype(NPBF16), "r3": r3, "cos1": cos1, "cw1": cw1,
            "sx": sx, "sxs": sxs, "gidx": gidx,
        })

    # ---- replicated params
    gm_w1 = np.asarray(inputs["gm_w1"], np.float32)
    pe_w1 = np.asarray(inputs["pe_w1"], np.float32)
    pe_w2 = np.asarray(inputs["pe_w2"], np.float32)
    pe_w3 = np.asarray(inputs["pe_w3"], np.float32)
    pe_b1 = np.asarray(inputs["pe_b1"], np.float32)
    pe_b2 = np.asarray(inputs["pe_b2"], np.float32)
    pe_b3 = np.asarray(inputs["pe_b3"], np.float32)
    w1ab = pe_w1[0:64, :]
    w1abD = np.concatenate([w1ab, w1ab], axis=1)  # [64, 128]
    w1cA = np.concatenate([pe_w1[64:96, :], pe_b1[None, :]], axis=0)  # [33,64]
    efT = ef.T  # [32, 80]
    efA = np.concatenate(
        [np.concatenate([efT[:, 0::2], efT[:, 1::2]], axis=1),
         np.ones((1, NE), np.float32)], axis=0)  # [33, 80]
    w2bd = np.zeros((128, 128), np.float32)
    w2bd[0:64, 0:64] = pe_w2
    w2bd[64:128, 64:128] = pe_w2
    w3bd = np.zeros((128, 128), np.float32)
    w3bd[0:64, 0:64] = pe_w3
    w3bd[64:128, 64:128] = pe_w3
    blk96 = np.zeros((3, 96), np.float32)
    for k in range(3):
        blk96[k, 32 * k:32 * (k + 1)] = 1.0
    wbf = np.zeros((128, 1104), np.float32)
    cc = [0]

    def put(arr, t):
        r, k = arr.shape
        t[0:r, cc[0]:cc[0] + k] = arr
        cc[0] += k
    put(gm_w1[0:128, :], wbf)
    put(gm_w1[128:256, :], wbf)
    put(gm_w1[256:353, :], wbf)
    put(np.asarray(inputs["gm_w2"], np.float32), wbf)
    put(np.asarray(inputs["gm_w3"], np.float32), wbf)
    put(w2bd, wbf)
    put(w3bd, wbf)
    put(w1abD, wbf)
    put(w1cA, wbf)
    put(efA, wbf)
    wfr = np.zeros((128, 352), np.float32)
    cc = [0]
    put(np.asarray(inputs["op_w1"], np.float32), wfr)
    put(np.asarray(inputs["op_w2"], np.float32), wfr)
    put(blk96, wfr)
    put(np.ones((1, 64), np.float32), wfr)
    wf2 = np.zeros((128, 208), np.float32)
    cc = [0]
    put(np.asarray(inputs["gm_b1"], np.float32)[:, None], wf2)
    put(np.asarray(inputs["gm_b2"], np.float32)[:, None], wf2)
    put(np.asarray(inputs["gm_b3"], np.float32)[:, None], wf2)
    put(np.concatenate([pe_b2, pe_b2])[:, None].astype(np.float32), wf2)
    put(np.concatenate([pe_b3, pe_b3])[:, None].astype(np.float32), wf2)
    put(-np.tile(np.arange(RBF, dtype=np.float32), 3)[:, None], wf2)
    put(np.asarray(inputs["op_b1"], np.float32)[:, None], wf2)
    put(np.asarray(inputs["op_b2"], np.float32)[:, None], wf2)
    put(np.ones((1, 128), np.float32), wf2)
    put(np.eye(64, dtype=np.float32), wf2)
    params = {
        "wbf": wbf.astype(NPBF16),
        "wfr": wfr,
        "wf2": wf2,
    }
    for m in in_maps:
        m.update(params)
    return T, use_sum, in_maps


def _ensure_ntff_hook():
    """Inject antenv.axon_hooks (missing in this image) so trace=True works."""
    try:
        from antenv.axon_hooks import get_axon_ntff_profile_hook  # noqa: F401
        return
    except ImportError:
        pass
    import sys
    import types

    import antenv
    mod = types.ModuleType("antenv.axon_hooks")
    mod._hook = None
    mod.set_axon_ntff_profile_hook = lambda h: setattr(mod, "_hook", h)
    mod.get_axon_ntff_profile_hook = lambda: mod._hook
    sys.modules["antenv.axon_hooks"] = mod
    antenv.axon_hooks = mod
    try:
        from trn_agent_boot.trn_boot import _ntff_profile_via_ctypes
        mod._hook = _ntff_profile_via_ctypes("/opt/axon/libaxon_pjrt.so")
    except Exception as e:  # degrade to no-trace
        print("ntff hook setup failed:", e)


def kernel(**inputs) -> np.ndarray:
    T, use_sum, in_maps = _prep(inputs)
    nc = _get_nc(T, use_sum)
    trace = bool(int(os.environ.get("KERNEL_TRACE", "0")))
    if trace:
        _ensure_ntff_hook()
        import concourse.bass_utils as _bu
        _bu.upload_artifacts = lambda d: "local"
    res = run_bass_kernel_spmd(nc, in_maps, list(range(NCORES)), trace=trace,
                               tmpdir=os.environ.get("KERNEL_TRACE_DIR"))
    global LAST_RESULTS
    LAST_RESULTS = res
    out = np.empty((B, NE, S), np.float32)
    for c in range(NCORES):
        oc = np.asarray(res.results[c]["out"], np.float32).reshape(BL, NE, S)
        for bl in range(BL):
            out[BL * c + bl] = oc[bl]
    return out


LAST_RESULTS = None


# revision 14
# speedup vs baseline: 1.1591x; 1.0991x over previous
"""Trainium2 Bass kernel for nn_AbsorberPathAggregator (v2).

Strategy: host-side path filtering (cutoff weight == 0 exactly for ~42% of
paths -> zero contribution, dropped exactly), survivors packed into 64-path
half-tiles batch-padded to 64, half-tiles distributed so each (core, batch)
owns at most one (seed-stable; a sum-merge variant handles up to two).

Device pipeline per core (W = T*128 path columns, T ~ 5):
  prepass: u = w1ab^T [ej;ek] (one matmul), v(e)+b1 (one matmul), rbf via
    broadcast-matmul + Square/Exp, geom MLP in bf16, gg2 = cw*(g3+b3) on DVE.
  e-loop over 40 folded e-pairs, software-pipelined across engines:
    scalar : h1 = silu(u (+) v_e)   -- the add rides the ACTIVATE bias port
    tensor : L2p = w2bd @ h1        (bf16, 640-wide moving)
    scalar : h2 = silu(L2p + b2)
    tensor : L3p = w3bd @ h2
    vector : contrib = L3p * gg2 ; slot cols = reduce_f(contrib) per half-tile
  tail: indirect-scatter slot blocks (bf16) into a batch-indexed DRAM
    accumulator, one bf16 AllReduce, indirect-gather this core's 2 batches,
    then the out-projection and a direct store.
"""

import os

import numpy as np
import ml_dtypes

import concourse.bacc as bacc
import concourse.bass as bass
import concourse.mybir as mybir
import concourse.tile as tile
from concourse.bass_utils import run_bass_kernel_spmd

F32 = mybir.dt.float32
F32R = mybir.dt.float32r
BF16 = mybir.dt.bfloat16
I32 = mybir.dt.int32
I16 = mybir.dt.int16
NPBF16 = ml_dtypes.bfloat16

NCORES = 8
B = 16
BL = 2              # batches per core after ReduceScatter
NE = 80
S = 64
EP = NE // 2        # folded e-pairs
HF = 64             # paths per half-tile
SLOTC = 42          # 40 agg cols + Sgg col + norm col
ATOM = 128
RBF = 32
CUT = 5.0
RBF_SCALE = (RBF - 1) / CUT

_NC_CACHE = {}


def _bc_last(ap, n):
    """[...dims] -> [...dims, n] with 0-step last dim."""
    l = [list(x) for x in ap.ap]
    return bass.AP(ap.tensor, ap.offset, l + [[0, n]])


def build_nc(T: int, use_sum: bool) -> bass.Bass:
    nc = bacc.Bacc("TRN2", target_bir_lowering=False, debug=False,
                   num_devices=NCORES)
    AF = mybir.ActivationFunctionType
    ALU = mybir.AluOpType
    T2 = 2 * T
    W = T2 * HF

    def wch():
        out, k = [], 0
        while k < W:
            out.append((k, min(k + 512, W)))
            k += 512
        return out

    # ---- per-core inputs
    hjT_d = nc.dram_tensor("hjT", [ATOM, W], BF16, kind="ExternalInput")
    hkT_d = nc.dram_tensor("hkT", [ATOM, W], BF16, kind="ExternalInput")
    ejk_d = nc.dram_tensor("ejk", [64, W], BF16, kind="ExternalInput")
    r3_d = nc.dram_tensor("r3", [3, W], F32R, kind="ExternalInput")
    cos1_d = nc.dram_tensor("cos1", [1, W], F32, kind="ExternalInput")
    cw1_d = nc.dram_tensor("cw1", [1, W], F32R, kind="ExternalInput")
    sx_d = nc.dram_tensor("sx", [128, T2], I32, kind="ExternalInput")
    sxs_d = nc.dram_tensor("sxs", [128, T], I32, kind="ExternalInput")
    gidx_d = nc.dram_tensor("gidx", [128, BL], I32, kind="ExternalInput")
    gix_d = nc.dram_tensor("gix", [128, 1], I16, kind="ExternalInput")
    # ---- packed replicated params (one DMA each)
    wbf_d = nc.dram_tensor("wbf", [128, 1104], BF16, kind="ExternalInput")
    wfr_d = nc.dram_tensor("wfr", [128, 352], F32R, kind="ExternalInput")
    wf2_d = nc.dram_tensor("wf2", [128, 208], F32, kind="ExternalInput")
    # ---- output + collective buffers
    out_d = nc.dram_tensor("out", [BL * NE, S], F32, kind="ExternalOutput")
    aggin_d = nc.dram_tensor("aggin", [(B + 1) * 128, SLOTC], BF16)
    aggout_d = nc.dram_tensor("aggout", [B * 128, SLOTC], BF16,
                              addr_space="Shared")

    with tile.TileContext(nc) as tc:
        with tc.tile_pool(name="const", bufs=1) as cp:
            def cl(dram, shape, dt, eng):
                t = cp.tile(shape, dt, tag=dram.name)
                eng.dma_start(t[:], dram[:])
                return t

            # ACT-table warmup: pull the Exp set in before anything needs it
            warm = cp.tile([1, 8], F32, tag="warm")
            nc.vector.memset(warm[:], 0.25)
            nc.scalar.activation(warm[0:1, 0:1], warm[0:1, 1:2], AF.Exp)

            # critical-path loads on the sync HWDGE ring
            ejk = cl(ejk_d, [64, W], BF16, nc.sync)
            wbf = cl(wbf_d, [128, 1104], BF16, nc.sync)
            r3 = cl(r3_d, [3, W], F32R, nc.sync)
            cw1 = cl(cw1_d, [1, W], F32R, nc.sync)
            hjA = cl(hjT_d, [ATOM, W], BF16, nc.sync)
            hkA = cl(hkT_d, [ATOM, W], BF16, nc.sync)
            # the rest via gpsimd SWDGE (its queue is idle here)
            wfr = cl(wfr_d, [128, 352], F32R, nc.gpsimd)
            wf2 = cl(wf2_d, [128, 208], F32, nc.gpsimd)
            cos1 = cl(cos1_d, [1, W], F32, nc.gpsimd)
            sx = cl(sx_d, [128, T2], I32, nc.gpsimd)
            gx = cl(gidx_d, [128, BL], I32, nc.gpsimd)
            gixt = cl(gix_d, [128, 1], I16, nc.gpsimd)
            sxs = cl(sxs_d, [128, T], I32, nc.gpsimd) if use_sum else None

            # views into the packed param tiles
            c = [0]

            def vw(t, rows, cols):
                a = t[0:rows, c[0]:c[0] + cols]
                c[0] += cols
                return a
            gw1a = vw(wbf, 128, 128)
            gw1b = vw(wbf, 128, 128)
            gw1c = vw(wbf, 97, 128)
            gw2 = vw(wbf, 128, 128)
            gw3 = vw(wbf, 128, 64)
            w2bd = vw(wbf, 128, 128)
            w3bd = vw(wbf, 128, 128)
            w1abD = vw(wbf, 64, 128)
            w1cA = vw(wbf, 33, 64)
            efA = vw(wbf, 33, NE)
            c = [0]
            ow1 = vw(wfr, 64, 128)
            ow2 = vw(wfr, 128, 64)
            blk96 = vw(wfr, 3, 96)
            ones64 = vw(wfr, 1, 64)
            c = [0]
            gb1 = vw(wf2, 128, 1)
            gb2 = vw(wf2, 128, 1)
            gb3 = vw(wf2, 64, 1)
            b2c2 = vw(wf2, 128, 1)
            b3c2 = vw(wf2, 128, 1)
            rb96 = vw(wf2, 96, 1)
            ob1 = vw(wf2, 128, 1)
            ob2 = vw(wf2, 64, 1)
            ones128 = vw(wf2, 1, 128)
            id64 = vw(wf2, 64, 64)


            with tc.tile_pool(name="keep", bufs=1) as kp:
                u2s = kp.tile([128, W], F32, tag="u2s")
                v2 = kp.tile([128, EP], F32, tag="v2")
                gg2 = kp.tile([128, W], F32, tag="gg2")
                slotAll = kp.tile([128, T2, SLOTC], F32, tag="slotAll")
                nc.vector.memset(slotAll[:, :, :], 0.0)

                with (
                    tc.tile_pool(name="pre", bufs=1) as pp,
                    tc.tile_pool(name="pps", bufs=1, space="PSUM") as pps,
                ):
                    # v2 = w1c^T ef + b1 (bias via augmented row)
                    vps = pps.tile([64, NE], F32, tag="pa")
                    nc.tensor.matmul(vps[:], w1cA[:], efA[:], start=True,
                                     stop=True)
                    nc.scalar.activation(v2[0:64, :], vps[:, 0:EP], AF.Copy)
                    nc.scalar.activation(v2[64:128, :], vps[:, EP:NE],
                                         AF.Copy)
                    # u2 = blockdup(w1ab)^T [ej; ek]  (both folds identical)
                    u2p = pps.tile([128, W], F32, tag="pb")
                    for k0, k1 in wch():
                        nc.tensor.matmul(u2p[:, k0:k1], w1abD[:],
                                         ejk[:, k0:k1], start=True, stop=True)
                    nc.scalar.activation(u2s[:], u2p[:], AF.Copy)
                    # rbf features
                    r96p = pps.tile([96, W], F32, tag="pb")
                    for k0, k1 in wch():
                        nc.tensor.matmul(r96p[:, k0:k1], blk96[:],
                                         r3[:, k0:k1], start=True, stop=True)
                    sq96 = pp.tile([96, W], F32, tag="sq96")
                    nc.scalar.activation(sq96[:], r96p[:], AF.Square,
                                         bias=rb96[:], scale=RBF_SCALE)
                    grbf = pp.tile([97, W], BF16, tag="grbf")
                    nc.scalar.activation(grbf[0:96, :], sq96[:], AF.Exp,
                                         scale=-0.5)
                    nc.vector.tensor_copy(grbf[96:97, :], cos1[:])
                    # geom MLP
                    gp = pps.tile([128, W], F32, tag="pa")
                    for k0, k1 in wch():
                        nc.tensor.matmul(gp[:, k0:k1], gw1a[:], hjA[:, k0:k1],
                                         start=True, stop=False)
                        nc.tensor.matmul(gp[:, k0:k1], gw1b[:], hkA[:, k0:k1],
                                         start=False, stop=False)
                        nc.tensor.matmul(gp[:, k0:k1], gw1c[:],
                                         grbf[:, k0:k1], start=False,
                                         stop=True)
                    h1g = pp.tile([128, W], BF16, tag="h1g")
                    nc.scalar.activation(h1g[:], gp[:], AF.Silu, bias=gb1[:])
                    gp2 = pps.tile([128, W], F32, tag="pb")
                    for k0, k1 in wch():
                        nc.tensor.matmul(gp2[:, k0:k1], gw2[:], h1g[:, k0:k1],
                                         start=True, stop=True)
                    h2g = pp.tile([128, W], BF16, tag="h2g")
                    nc.scalar.activation(h2g[:], gp2[:], AF.Silu, bias=gb2[:])
                    g3p = pps.tile([64, W], F32, tag="pa")
                    for k0, k1 in wch():
                        nc.tensor.matmul(g3p[:, k0:k1], gw3[:], h2g[:, k0:k1],
                                         start=True, stop=True)
                    # gg2 = cw * (g3 + gb3), duplicated on both folds
                    cwB = pps.tile([64, W], F32, tag="pc")
                    for k0, k1 in wch():
                        nc.tensor.matmul(cwB[:, k0:k1], ones64[:],
                                         cw1[:, k0:k1], start=True, stop=True)
                    t3 = pp.tile([64, W], F32, tag="t3")
                    nc.vector.tensor_scalar_add(t3[:], g3p[:], gb3[:])
                    nc.vector.tensor_tensor(gg2[0:64, :], t3[:], cwB[:],
                                            op=ALU.mult)
                    nc.vector.tensor_copy(gg2[64:128, :], gg2[0:64, :])
                    # Sgg and norm slot columns
                    gg2v = gg2[:, :].rearrange("p (t f) -> p t f", t=T2)
                    nc.vector.tensor_reduce(slotAll[:, :, 40:41], gg2v,
                                            axis=mybir.AxisListType.X,
                                            op=ALU.add)
                    cw1v = cw1[:, :].rearrange("p (t f) -> p t f", t=T2)
                    nc.vector.tensor_reduce(slotAll[0:1, :, 41:42], cw1v,
                                            axis=mybir.AxisListType.X,
                                            op=ALU.add)

                # ---- e-pair loop, software pipelined
                with (
                    tc.tile_pool(name="ph1", bufs=3) as ph1,
                    tc.tile_pool(name="ph2", bufs=3) as ph2,
                    tc.tile_pool(name="pco", bufs=2) as pco,
                    tc.tile_pool(name="psL", bufs=2, space="PSUM") as psL,
                ):
                    h1_t = [None] * EP
                    l2_t = [None] * EP

                    def post_h1(e):
                        t = ph1.tile([128, W], BF16, tag="h1")
                        nc.scalar.activation(t[:], u2s[:], AF.Silu,
                                             bias=v2[:, e:e + 1])
                        h1_t[e] = t

                    def post_l2(e):
                        t = psL.tile([128, W], F32, tag="l2")
                        for k0, k1 in wch():
                            nc.tensor.matmul(t[:, k0:k1], w2bd[:],
                                             h1_t[e][:, k0:k1], start=True,
                                             stop=True)
                        h1_t[e] = None
                        l2_t[e] = t

                    post_h1(0)
                    post_l2(0)
                    for e in range(EP):
                        if e + 1 < EP:
                            post_h1(e + 1)
                        h2 = ph2.tile([128, W], BF16, tag="h2")
                        nc.scalar.activation(h2[:], l2_t[e][:], AF.Silu,
                                             bias=b2c2[:])
                        l2_t[e] = None
                        if e + 1 < EP:
                            post_l2(e + 1)
                        l3 = psL.tile([128, W], F32, tag="l3")
                        for k0, k1 in wch():
                            nc.tensor.matmul(l3[:, k0:k1], w3bd[:],
                                             h2[:, k0:k1], start=True,
                                             stop=True)
                        co = pco.tile([128, W], F32, tag="co")
                        nc.vector.tensor_tensor(co[:], l3[:], gg2[:],
                                                op=ALU.mult)
                        cov = co[:, :].rearrange("p (t f) -> p t f", t=T2)
                        nc.vector.tensor_reduce(slotAll[:, :, e:e + 1], cov,
                                                axis=mybir.AxisListType.X,
                                                op=ALU.add)

                # ---- fold b3*Sgg into the agg columns, cast to bf16
                sggb = kp.tile([128, T2], F32, tag="sggb")
                nc.vector.tensor_scalar(sggb[:, :], slotAll[:, :, 40],
                                        b3c2[:], None, op0=ALU.mult)
                nc.vector.tensor_tensor(slotAll[:, :, 0:40],
                                        slotAll[:, :, 0:40],
                                        _bc_last(sggb[:, :], 40), op=ALU.add)
                slotBf = kp.tile([128, T2 + 1, SLOTC], BF16, tag="slotBf")
                nc.vector.memset(slotBf[:, T2:T2 + 1, :], 0.0)
                nc.vector.tensor_copy(slotBf[:, 0:T2, :], slotAll[:, :, :])
                if use_sum:
                    slotS = kp.tile([128, T, SLOTC], BF16, tag="slotS")
                    nc.vector.tensor_tensor(slotS[:, :, :],
                                            slotAll[:, 0::2, :],
                                            slotAll[:, 1::2, :], op=ALU.add)
                    for _t in range(T2):
                        nc.gpsimd.indirect_dma_start(
                            out=aggin_d[:, :],
                            out_offset=bass.IndirectOffsetOnAxis(
                                ap=sx[:, _t:_t + 1], axis=0),
                            in_=slotBf[:, _t, :],
                            in_offset=None,
                        )
                    for _t in range(T):
                        nc.gpsimd.indirect_dma_start(
                            out=aggin_d[:, :],
                            out_offset=bass.IndirectOffsetOnAxis(
                                ap=sxs[:, _t:_t + 1], axis=0),
                            in_=slotS[:, _t, :],
                            in_offset=None,
                        )
                else:
                    # SBUF column-gather into batch order, then one direct DMA
                    aggsb = kp.tile([128, B, SLOTC], BF16, tag="aggsb")
                    nc.gpsimd.ap_gather(aggsb[:, :, :], slotBf[:, :, :],
                                        gixt[:, :], channels=128,
                                        num_elems=T2 + 1, d=SLOTC, num_idxs=B)
                    nc.sync.dma_start(
                        bass.AP(aggin_d[:, :].tensor, 0,
                                [[SLOTC, 128], [128 * SLOTC, B], [1, SLOTC]]),
                        aggsb[:, :, :])
                nc.gpsimd.collective_compute(
                    "AllReduce",
                    mybir.AluOpType.add,
                    replica_groups=[list(range(NCORES))],
                    ins=[aggin_d[0:B * 128, :]],
                    outs=[aggout_d[:, :]],
                )

            # ---- endgame: normalize + out-MLP on this core's 2 batches
            with (
                tc.tile_pool(name="eg", bufs=1) as eg,
                tc.tile_pool(name="egp", bufs=2, space="PSUM") as egp,
            ):
                agg2 = eg.tile([128, BL, SLOTC], BF16, tag="agg2")
                for bl in range(BL):
                    nc.gpsimd.indirect_dma_start(
                        out=agg2[:, bl, :],
                        out_offset=None,
                        in_=aggout_d[:, :],
                        in_offset=bass.IndirectOffsetOnAxis(
                            ap=gx[:, bl:bl + 1], axis=0),
                    )
                norm2 = eg.tile([1, BL], F32, tag="norm2")
                nc.vector.tensor_scalar_max(norm2[:], agg2[0:1, :, 41], 1e-8)
                rn = eg.tile([1, BL], F32, tag="rn")
                nc.vector.reciprocal(rn[:], norm2[:])
                rnp = egp.tile([128, BL], F32, tag="rnp")
                nc.tensor.matmul(rnp[:], ones128[:], rn[:], start=True,
                                 stop=True)
                agn = eg.tile([128, BL, EP], F32, tag="agn")
                nc.vector.tensor_tensor(agn[:], agg2[:, :, 0:40],
                                        _bc_last(rnp[:, :], EP), op=ALU.mult)
                unf = eg.tile([64, BL, NE], F32R, tag="unf")
                nc.vector.tensor_copy(unf[:, :, 0::2], agn[0:64, :, :])
                nc.vector.tensor_copy(unf[:, :, 1::2], agn[64:128, :, :])

                NCOL = BL * NE
                unf_f = unf[:, :, :].rearrange("p a b -> p (a b)")
                hop = egp.tile([128, NCOL], F32, tag="hop")
                nc.tensor.matmul(hop[:], ow1[:], unf_f[:], start=True,
                                 stop=True)
                ho = eg.tile([128, BL, NE], F32R, tag="ho")
                ho_f = ho[:, :, :].rearrange("p a b -> p (a b)")
                nc.scalar.activation(ho_f[:], hop[:], AF.Silu, bias=ob1[:])
                o2p = egp.tile([64, NCOL], F32, tag="o2p")
                nc.tensor.matmul(o2p[:], ow2[:], ho_f[:], start=True,
                                 stop=True)
                outf = eg.tile([64, BL, NE], F32, tag="outf")
                outf_f = outf[:, :, :].rearrange("p a b -> p (a b)")
                nc.vector.tensor_scalar_add(outf_f[:], o2p[:], ob2[:])
                t80 = eg.tile([NE, BL, S], F32, tag="t80")
                for bl in range(BL):
                    tp = egp.tile([NE, S], F32, tag="tp")
                    nc.tensor.transpose(tp[:], outf[:, bl, :], id64[:])
                    nc.scalar.copy(t80[:, bl, :], tp[:])
                    nc.sync.dma_start(out_d[bl * NE:(bl + 1) * NE, :],
                                      t80[:, bl, :])
    nc.compile()
    return nc


def _get_nc(T, use_sum):
    if (T, use_sum) not in _NC_CACHE:
        _NC_CACHE[(T, use_sum)] = build_nc(T, use_sum)
    return _NC_CACHE[(T, use_sum)]


def _cutoff(r):
    return np.where(r < CUT,
                    0.5 * (np.cos(np.pi * np.minimum(r, CUT) / CUT) + 1.0),
                    0.0).astype(np.float32)


def _prep(inputs):
    h = np.asarray(inputs["h_flat"], dtype=np.float32)
    z = np.asarray(inputs["z_flat"]).astype(np.int64)
    ef = np.asarray(inputs["e_feat"], dtype=np.float32)
    pj = np.asarray(inputs["path_j"]).astype(np.int64)
    pk = np.asarray(inputs["path_k"]).astype(np.int64)
    r0j = np.asarray(inputs["path_r0j"], dtype=np.float32)
    r0k = np.asarray(inputs["path_r0k"], dtype=np.float32)
    rjk = np.asarray(inputs["path_rjk"], dtype=np.float32)
    cosa = np.asarray(inputs["path_cosangle"], dtype=np.float32)
    pb = np.asarray(inputs["path_batch"]).astype(np.int64)
    zemb = np.asarray(inputs["z_emb"], dtype=np.float32)
    assert int(inputs["bsz"]) == B

    cw = _cutoff(r0j) * _cutoff(r0k) * _cutoff(rjk)
    keep = (r0j < CUT) & (r0k < CUT) & (rjk < CUT)
    order = np.argsort(pb, kind="stable")
    order = order[keep[order]]

    # half-tiles of <= 64 paths, per batch
    halves = []
    for b in range(B):
        idxs = order[pb[order] == b]
        for k0 in range(0, len(idxs), HF):
            halves.append((b, idxs[k0:k0 + HF]))

    # distribute: at most one half-tile per (core, batch) when possible
    core_halves = [[] for _ in range(NCORES)]
    core_batches = [dict() for _ in range(NCORES)]  # batch -> count
    rr = 0
    overflow = []
    for (b, idxs) in halves:
        placed = False
        for k in range(NCORES):
            c = (rr + k) % NCORES
            if b not in core_batches[c]:
                core_halves[c].append((b, idxs))
                core_batches[c][b] = 1
                rr = (c + 1) % NCORES
                placed = True
                break
        if not placed:
            overflow.append((b, idxs))
    use_sum = len(overflow) > 0
    merged = [dict() for _ in range(NCORES)]  # core -> {tile_idx: batch}
    if use_sum:
        # place each overflow half next to its same-batch sibling as the
        # two halves of one tile; that tile scatters only its summed block
        for (b, idxs) in overflow:
            done = False
            for c in range(NCORES):
                if core_batches[c].get(b, 0) == 1:
                    j = next(i for i, (bb, _) in enumerate(core_halves[c])
                             if bb == b)
                    lst = core_halves[c]
                    # move sibling to an even index at the end, pair them
                    sib = lst.pop(j)
                    if len(lst) % 2 == 1:
                        lst.append((None, np.empty(0, np.int64)))
                    ti = len(lst) // 2
                    lst.append(sib)
                    lst.append((b, idxs))
                    core_batches[c][b] = 2
                    merged[c][ti] = b
                    done = True
                    break
            assert done, "batch needs >2 half-tiles on one core"

    T = max(1, max((len(ch) + 1) // 2 for ch in core_halves))
    T2 = 2 * T
    W = T2 * HF
    hT = h.T  # (128, 1024)
    ezT = zemb.T  # (32, 101)

    in_maps = []
    for c in range(NCORES):
        ch = list(core_halves[c])
        while len(ch) < T2:
            ch.append((None, np.empty(0, np.int64)))
        hjT = np.zeros((ATOM, W), np.float32)
        hkT = np.zeros((ATOM, W), np.float32)
        ejk = np.zeros((64, W), np.float32)
        r3 = np.full((3, W), CUT, np.float32)
        cos1 = np.zeros((1, W), np.float32)
        cw1 = np.zeros((1, W), np.float32)
        sx = np.empty((128, T2), np.int32)
        sxs = np.full((128, T), B * 128, np.int32)
        sxs += np.arange(128, dtype=np.int32)[:, None]
        for j, (b, idxs) in enumerate(ch):
            ti = j // 2
            if merged[c].get(ti) is not None and b is not None:
                # merged tile: halves scatter to trash, sum goes to batch
                sx[:, j] = B * 128 + np.arange(128)
                if j % 2 == 0:
                    sxs[:, ti] = b * 128 + np.arange(128)
            elif b is None:
                sx[:, j] = B * 128 + np.arange(128)
            else:
                sx[:, j] = b * 128 + np.arange(128)
            n = len(idxs)
            if n == 0:
                continue
            cols = slice(j * HF, j * HF + n)
            hjT[:, cols] = hT[:, pj[idxs]]
            hkT[:, cols] = hT[:, pk[idxs]]
            ejk[0:32, cols] = ezT[:, z[pj[idxs]]]
            ejk[32:64, cols] = ezT[:, z[pk[idxs]]]
            r3[0, cols] = np.minimum(r0j[idxs], CUT)
            r3[1, cols] = np.minimum(r0k[idxs], CUT)
            r3[2, cols] = np.minimum(rjk[idxs], CUT)
            cos1[0, cols] = cosa[idxs]
            cw1[0, cols] = cw[idxs]
        gidx = np.empty((128, BL), np.int32)
        for bl in range(BL):
            gidx[:, bl] = (BL * c + bl) * 128 + np.arange(128)
        inv = np.full(B, 2 * T, np.int16)
        for j, (b, idxs) in enumerate(ch):
            if b is not None and len(idxs) > 0 and merged[c].get(j // 2) is None:
                inv[b] = j
        gix = np.empty((128, 1), np.int16)
        gix[:, 0] = inv[np.arange(128) % 16]
        in_maps.append({
            "hjT": hjT.astype(NPBF16), "hkT": hkT.astype(NPBF16),
            "ejk": ejk.astype(NPBF16), "r3": r3, "cos1": cos1, "cw1": cw1,
            "sx": sx, "sxs": sxs, "gidx": gidx, "gix": gix,
        })

    # ---- replicated params
    gm_w1 = np.asarray(inputs["gm_w1"], np.float32)
    pe_w1 = np.asarray(inputs["pe_w1"], np.float32)
    pe_w2 = np.asarray(inputs["pe_w2"], np.float32)
    pe_w3 = np.asarray(inputs["pe_w3"], np.float32)
    pe_b1 = np.asarray(inputs["pe_b1"], np.float32)
    pe_b2 = np.asarray(inputs["pe_b2"], np.float32)
    pe_b3 = np.asarray(inputs["pe_b3"], np.float32)
    w1ab = pe_w1[0:64, :]
    w1abD = np.concatenate([w1ab, w1ab], axis=1)  # [64, 128]
    w1cA = np.concatenate([pe_w1[64:96, :], pe_b1[None, :]], axis=0)  # [33,64]
    efT = ef.T  # [32, 80]
    efA = np.concatenate(
        [np.concatenate([efT[:, 0::2], efT[:, 1::2]], axis=1),
         np.ones((1, NE), np.float32)], axis=0)  # [33, 80]
    w2bd = np.zeros((128, 128), np.float32)
    w2bd[0:64, 0:64] = pe_w2
    w2bd[64:128, 64:128] = pe_w2
    w3bd = np.zeros((128, 128), np.float32)
    w3bd[0:64, 0:64] = pe_w3
    w3bd[64:128, 64:128] = pe_w3
    blk96 = np.zeros((3, 96), np.float32)
    for k in range(3):
        blk96[k, 32 * k:32 * (k + 1)] = 1.0
    wbf = np.zeros((128, 1104), np.float32)
    cc = [0]

    def put(arr, t):
        r, k = arr.shape
        t[0:r, cc[0]:cc[0] + k] = arr
        cc[0] += k
    put(gm_w1[0:128, :], wbf)
    put(gm_w1[128:256, :], wbf)
    put(gm_w1[256:353, :], wbf)
    put(np.asarray(inputs["gm_w2"], np.float32), wbf)
    put(np.asarray(inputs["gm_w3"], np.float32), wbf)
    put(w2bd, wbf)
    put(w3bd, wbf)
    put(w1abD, wbf)
    put(w1cA, wbf)
    put(efA, wbf)
    wfr = np.zeros((128, 352), np.float32)
    cc = [0]
    put(np.asarray(inputs["op_w1"], np.float32), wfr)
    put(np.asarray(inputs["op_w2"], np.float32), wfr)
    put(blk96, wfr)
    put(np.ones((1, 64), np.float32), wfr)
    wf2 = np.zeros((128, 208), np.float32)
    cc = [0]
    put(np.asarray(inputs["gm_b1"], np.float32)[:, None], wf2)
    put(np.asarray(inputs["gm_b2"], np.float32)[:, None], wf2)
    put(np.asarray(inputs["gm_b3"], np.float32)[:, None], wf2)
    put(np.concatenate([pe_b2, pe_b2])[:, None].astype(np.float32), wf2)
    put(np.concatenate([pe_b3, pe_b3])[:, None].astype(np.float32), wf2)
    put(-np.tile(np.arange(RBF, dtype=np.float32), 3)[:, None], wf2)
    put(np.asarray(inputs["op_b1"], np.float32)[:, None], wf2)
    put(np.asarray(inputs["op_b2"], np.float32)[:, None], wf2)
    put(np.ones((1, 128), np.float32), wf2)
    put(np.eye(64, dtype=np.float32), wf2)
    params = {
        "wbf": wbf.astype(NPBF16),
        "wfr": wfr,
        "wf2": wf2,
    }
    for m in in_maps:
        m.update(params)
    return T, use_sum, in_maps


def _ensure_ntff_hook():
    """Inject antenv.axon_hooks (missing in this image) so trace=True works."""
    try:
        from antenv.axon_hooks import get_axon_ntff_profile_hook  # noqa: F401
        return
    except ImportError:
        pass
    import sys
    import types

    import antenv
    mod = types.ModuleType("antenv.axon_hooks")
    mod._hook = None
    mod.set_axon_ntff_profile_hook = lambda h: setattr(mod, "_hook", h)
    mod.get_axon_ntff_profile_hook = lambda: mod._hook
    sys.modules["antenv.axon_hooks"] = mod
    antenv.axon_hooks = mod
    try:
        from trn_agent_boot.trn_boot import _ntff_profile_via_ctypes
        mod._hook = _ntff_profile_via_ctypes("/opt/axon/libaxon_pjrt.so")
    except Exception as e:  # degrade to no-trace
        print("ntff hook setup failed:", e)


def kernel(**inputs) -> np.ndarray:
    T, use_sum, in_maps = _prep(inputs)
    nc = _get_nc(T, use_sum)
    trace = bool(int(os.environ.get("KERNEL_TRACE", "0")))
    if trace:
        _ensure_ntff_hook()
        import concourse.bass_utils as _bu
        _bu.upload_artifacts = lambda d: "local"
    res = run_bass_kernel_spmd(nc, in_maps, list(range(NCORES)), trace=trace,
                               tmpdir=os.environ.get("KERNEL_TRACE_DIR"))
    global LAST_RESULTS
    LAST_RESULTS = res
    out = np.empty((B, NE, S), np.float32)
    for c in range(NCORES):
        oc = np.asarray(res.results[c]["out"], np.float32).reshape(BL, NE, S)
        for bl in range(BL):
            out[BL * c + bl] = oc[bl]
    return out


LAST_RESULTS = None
